# revision 1
# baseline (speedup 1.0000x reference)
"""GAT (2-layer, 4/1 heads) on 8 trn2 NeuronCores via Bass/Tile.

Strategy (dst-partitioned, gather-based):
- Edges (+self loops) sorted by dst; each core owns a contiguous dst range
  (N/8 nodes). Per 128-dst window, the segment softmax+sum is computed via
  one-hot matmuls accumulating in PSUM.
- Per-node records (h plus attention s-values) live in a DRAM table with
  256B-multiple row stride; per-edge rows are fetched with InstDMAGatherAnt
  (int16 indices relative to a src-quarter base). Per-edge d-values come
  from a per-core-local d table (dst-local indices, dummy row for padding).
- Node tables are built per-core for the core's node slice, then AllGather'd.
- Final: graph mean-pool via one-hot matmuls, AllReduce, tiny linear.
"""

import math

import numpy as np
import ml_dtypes

import concourse.bass as bass
import concourse.mybir as mybir
import concourse.tile as tile
from concourse import bacc
from concourse.bass_utils import run_bass_kernel_spmd
from concourse.masks import make_identity

NCORES = 8
P = 128
NEG_SLOPE = 0.2
BWIN = 6          # windows per batch
GRP = 4           # tiles per O/rhs build group

bf16 = mybir.dt.bfloat16
f32 = mybir.dt.float32
i16 = mybir.dt.int16

_last_exec_ns = None


def _install_ntff_hook():
    """Provide antenv.axon_hooks (missing on this image) so trace=True works."""
    import sys
    import types
    try:
        from antenv import axon_hooks  # noqa: F401
        return
    except ImportError:
        pass
    import antenv
    mod = types.ModuleType("antenv.axon_hooks")
    mod._hook = None
    mod.set_axon_ntff_profile_hook = lambda h: setattr(mod, "_hook", h)
    mod.get_axon_ntff_profile_hook = lambda: mod._hook
    sys.modules["antenv.axon_hooks"] = mod
    antenv.axon_hooks = mod
    try:
        from trn_agent_boot.trn_boot import _ntff_profile_via_ctypes
        mod._hook = _ntff_profile_via_ctypes("/opt/axon/libaxon_pjrt.so")
    except Exception:
        mod._hook = None
    # avoid remote artifact uploads in the trace path (sandbox is zero-egress)
    import concourse.bass_utils as bu
    bu.upload_artifacts = lambda tmpdir: f"local:{tmpdir}"


# ---------------------------------------------------------------- host helpers

def _wrap16(flat, pad_val=0):
    """int16 index list -> [128, ceil(n/16)] wrapped+replicated layout."""
    n = len(flat)
    cols = (n + 15) // 16
    a = np.full(cols * 16, pad_val, np.int16)
    a[:n] = flat
    w = a.reshape(cols, 16).T  # [16, cols]
    return np.tile(w, (8, 1))  # [128, cols]


def _slotmajor(flat, T, dtype):
    """slot-stream [T*128] -> [128, T] (slot i -> partition i%128, block i//128)."""
    return np.ascontiguousarray(flat.reshape(T, P).T.astype(dtype))


class Meta:
    pass


def _host_prep(x, edge_index, batch, heads, hid):
    N = x.shape[0]
    assert N % NCORES == 0
    NPC = N // NCORES
    NT = (NPC + P - 1) // P          # node tiles / windows per core
    NPC_pad = NT * P
    QN = (N + 3) // 4                # src quarter size
    assert QN + 256 < 32768, "quarter too big for int16 gather indices"
    assert NPC_pad + 16 < 32768

    E0 = edge_index.shape[1]
    src = np.concatenate([np.asarray(edge_index[0]), np.arange(N)]).astype(np.int64)
    dst = np.concatenate([np.asarray(edge_index[1]), np.arange(N)]).astype(np.int64)
    order = np.argsort(dst, kind="stable")
    src, dst = src[order], dst[order]

    core_edges = []
    for c in range(NCORES):
        lo = np.searchsorted(dst, c * NPC, "left")
        hi = np.searchsorted(dst, (c + 1) * NPC, "left")
        core_edges.append((src[lo:hi], dst[lo:hi]))

    NB = (NT + BWIN - 1) // BWIN     # batches
    # per (core, window, quarter) edge lists
    cell = [[[None] * 4 for _ in range(NT)] for _ in range(NCORES)]
    for c in range(NCORES):
        s_c, d_c = core_edges[c]
        w_of = (d_c - c * NPC) // P
        q_of = s_c // QN
        for w in range(NT):
            m = w_of == w
            sw, dw, qw = s_c[m], d_c[m], q_of[m]
            for q in range(4):
                mq = qw == q
                cell[c][w][q] = (sw[mq], dw[mq])

    # equalized tile counts per (window, quarter)
    Twq = np.zeros((NT, 4), np.int64)
    for w in range(NT):
        for q in range(4):
            mx = max(len(cell[c][w][q][0]) for c in range(NCORES))
            Twq[w, q] = (mx + P - 1) // P

    m = Meta()
    m.N, m.NPC, m.NT, m.NPC_pad, m.QN, m.NB = N, NPC, NT, NPC_pad, QN, NB
    m.heads, m.hid = heads, hid
    m.Twq = Twq
    # per-batch structure
    m.batches = []
    for b in range(NB):
        ws = list(range(b * BWIN, min((b + 1) * BWIN, NT)))
        Rq = [int(Twq[ws, q].sum()) for q in range(4)]
        Tb = sum(Rq)
        # block offset of (w, q): region base + windows before w in this region
        reg_base = np.cumsum([0] + Rq)[:4]
        blk = {}
        for q in range(4):
            off = reg_base[q]
            for w in ws:
                blk[(w, q)] = int(off)
                off += int(Twq[w, q])
        m.batches.append(dict(ws=ws, Rq=Rq, Tb=Tb, blk=blk, reg_base=reg_base))

    # per-core input arrays
    m.rec_cols = []   # per (b,q) col counts in rec_idx array
    per_core = []
    for c in range(NCORES):
        rec_idx_cols = []
        d_idx_cols = []
        li_cols = []
        for b in range(NB):
            B = m.batches[b]
            d_flat = np.zeros(B["Tb"] * P, np.int64)
            li_flat = np.zeros(B["Tb"] * P, np.int64)
            for q in range(4):
                r_flat = np.zeros(B["Rq"][q] * P, np.int64)
                for w in B["ws"]:
                    sw, dw = cell[c][w][q]
                    t0 = B["blk"][(w, q)]
                    nsl = int(Twq[w, q]) * P
                    sl = slice(t0 * P, t0 * P + nsl)
                    # pad: src->quarter base row 0 (w==0 via dummy d row)
                    rr = np.zeros(nsl, np.int64)
                    dd = np.full(nsl, NPC_pad, np.int64)  # dummy d row (-300)
                    ll = np.zeros(nsl, np.int64)
                    k = len(sw)
                    rr[:k] = sw - q * QN
                    dd[:k] = dw - c * NPC
                    ll[:k] = dw - (c * NPC + w * P)
                    r_flat[t0 * P - B["reg_base"][q] * P:
                           t0 * P - B["reg_base"][q] * P + nsl] = rr
                    d_flat[sl] = dd
                    li_flat[sl] = ll
                rec_idx_cols.append(_wrap16(r_flat.astype(np.int16)))
                if c == 0:
                    m.rec_cols.append(rec_idx_cols[-1].shape[1])
            d_idx_cols.append(_wrap16(d_flat.astype(np.int16)))
            li_cols.append(_slotmajor(li_flat, B["Tb"], np.float32))
        pc = dict(
            rec_idx=np.concatenate(rec_idx_cols, 1) if rec_idx_cols else
            np.zeros((P, 0), np.int16),
            d_idx=np.concatenate(d_idx_cols, 1),
            li=np.concatenate(li_cols, 1),
        )
        per_core.append(pc)
    m.d_cols = []
    m.li_cols = []
    for b in range(NB):
        m.d_cols.append((m.batches[b]["Tb"] * P + 15) // 16)
        m.li_cols.append(m.batches[b]["Tb"])

    # graph pooling metadata
    G = int(np.max(batch)) + 1
    m.G = G
    assert G <= 256
    counts = np.bincount(np.asarray(batch).astype(np.int64), minlength=256)
    recip = (1.0 / np.maximum(counts, 1)).astype(np.float32)
    m.recip = recip  # [256]
    for c in range(NCORES):
        gid = np.full(NPC_pad, -1, np.int64)
        gid[:NPC] = np.asarray(batch)[c * NPC:(c + 1) * NPC]
        gA = gid.astype(np.float64)
        gB = np.where(gid >= 0, gid - 128, -1).astype(np.float64)
        per_core[c]["gidA"] = _slotmajor(gA, NT, ml_dtypes.bfloat16)
        per_core[c]["gidB"] = _slotmajor(gB, NT, ml_dtypes.bfloat16)
    m.per_core = per_core
    return m


# ---------------------------------------------------------------- raw dma_gather

def _dma_gather_raw(gp, out_ap, in_ap, idxs_ap, num_idxs, elem_size, elem_step,
                    queue_num=0):
    """dma_gather without the elem%256B assert (stride must be 256B-mult)."""
    from concourse import ap_utils
    from concourse._compat import exact_div
    assert idxs_ap.dtype == i16
    assert in_ap.dtype == out_ap.dtype
    assert ap_utils.ap_is_contiguous(in_ap.ap[1:])
    assert ap_utils.ap_is_contiguous(out_ap.ap[1:])
    assert ap_utils.ap_is_contiguous(idxs_ap.ap[1:])
    assert in_ap.ap[0][0] == elem_step
    stride_bytes = elem_step * mybir.dt.size(in_ap.dtype)
    stride_256 = exact_div(stride_bytes, 256)
    assert stride_256 < 256
    _in_ap = gp.lower_ap_dma(in_ap, for_custom_bir_dma=True)
    _idxs_ap = gp.lower_ap(idxs_ap)
    _out_ap = gp.lower_ap(out_ap)
    return gp.add_instruction(
        mybir.InstDMAGatherAnt(
            name=gp.bass.get_next_instruction_name(),
            ins=[*_in_ap, _idxs_ap, gp.lower_val_access(gp.to_reg(num_idxs))],
            outs=[_out_ap],
            transpose=False,
            num_idxs=num_idxs,
            elem_size=elem_size,
            stride_bytes_256=stride_256,
            gen_mode=0,
            single_packet=True,
            queue_num=queue_num,
            sbuf_tokens_per_rank=0,
            sbuf_free_dim_per_rank=0,
            sbuf_free_dim_pad_per_rank=0,
            sbuf_byte_offset=0,
        )
    )


# ---------------------------------------------------------------- device program

def _build(m):
    nc = bacc.Bacc("TRN2", target_bir_lowering=False, debug=False,
                   num_devices=NCORES, num_swdge_queues=4)
    nc._swq = 0
    H, C = m.heads, m.hid
    HC = H * C                       # 128
    NPC_pad, NT, NB, QN = m.NPC_pad, m.NT, m.NB, m.QN
    R1 = HC + 2 * H                  # rec1 elem (bf16 units): h(128)+s_f32(8)
    R2 = C + 4                       # rec2 elem: h2(32)+one(1)+pad(1)+s2_f32(2)

    # ---------------- inputs
    def ein(name, shape, dt):
        return nc.dram_tensor(name, shape, dt, kind="ExternalInput")

    x_sl = ein("x_sl", [NPC_pad, HC], bf16)
    W1b = ein("W1b", [HC, HC], bf16)
    a1_bc = ein("a1_bc", [P, 2 * HC], bf16)      # [asrc1(128) | adst1(128)] rows replicated
    b1_bc = ein("b1_bc", [P, HC], f32)
    W2b = ein("W2b", [HC, C], bf16)
    a2_bc = ein("a2_bc", [P, 2 * C], bf16)
    b2_bc = ein("b2_bc", [P, C], f32)
    Wlin = ein("Wlin", [C, 10], f32)
    blin = ein("blin", [10, 1], f32)
    recip_in = ein("recip_in", [P, 2], f32)
    iota_bc = ein("iota_bc", [P, P], bf16)
    rec_idx = ein("rec_idx", [P, sum(m.rec_cols)], i16)
    d_idx = ein("d_idx", [P, sum(m.d_cols)], i16)
    li_in = ein("li_in", [P, sum(m.li_cols)], f32)
    gidA_in = ein("gidA", [P, NT], bf16)
    gidB_in = ein("gidB", [P, NT], bf16)

    out_t = nc.dram_tensor("out", [256, 10], f32, kind="ExternalOutput")

    # ---------------- internal DRAM
    cc1 = nc.dram_tensor("cc1", [NPC_pad, 2 * HC], bf16, kind="Internal")
    table1 = nc.dram_tensor("table1", [m.N + P, 2 * HC], bf16, kind="Internal",
                            addr_space="Shared")
    d1loc = nc.dram_tensor("d1loc", [NPC_pad + 16, 64], f32, kind="Internal")
    h1x = nc.dram_tensor("h1x", [NPC_pad, HC], bf16, kind="Internal")
    cc2 = nc.dram_tensor("cc2", [NPC_pad, P], bf16, kind="Internal")
    table2 = nc.dram_tensor("table2", [m.N + P, P], bf16, kind="Internal",
                            addr_space="Shared")
    d2loc = nc.dram_tensor("d2loc", [NPC_pad + 16, 64], f32, kind="Internal")
    hfin = nc.dram_tensor("hfin", [NPC_pad, C], bf16, kind="Internal")
    po_in = nc.dram_tensor("po_in", [256, C], f32, kind="Internal")
    po_out = nc.dram_tensor("po_out", [256, C], f32, kind="Internal")

    AL = mybir.AluOpType
    AF = mybir.ActivationFunctionType
    rg = [list(range(NCORES))]

    with tile.TileContext(nc) as tc:
        _phase0(nc, tc, m, x_sl, W1b, a1_bc, cc1, d1loc)
        nc.gpsimd.collective_compute(
            kind="AllGather", op=AL.bypass, replica_groups=rg,
            ins=[cc1[0:m.NPC, :]], outs=[table1[0:m.N, :]])
        _gat_layer(nc, tc, m, layer=1, table=table1, dloc=d1loc,
                   rec_elem=R1, s_off=HC, nh=H, ch=C, b_bc=b1_bc,
                   iota_bc=iota_bc, rec_idx=rec_idx, d_idx=d_idx,
                   li_in=li_in, out_dram=h1x)
        _phase2(nc, tc, m, h1x, W2b, a2_bc, cc2, d2loc)
        nc.gpsimd.collective_compute(
            kind="AllGather", op=AL.bypass, replica_groups=rg,
            ins=[cc2[0:m.NPC, :]], outs=[table2[0:m.N, :]])
        _gat_layer(nc, tc, m, layer=2, table=table2, dloc=d2loc,
                   rec_elem=R2, s_off=C + 2, nh=1, ch=C, b_bc=b2_bc,
                   iota_bc=iota_bc, rec_idx=rec_idx, d_idx=d_idx,
                   li_in=li_in, out_dram=hfin)
        _pool_final(nc, tc, m, hfin, gidA_in, gidB_in, iota_bc, recip_in,
                    Wlin, blin, po_in, po_out, out_t, rg)

    nc.compile()
    return nc


def _phase0(nc, tc, m, x_sl, W1b, a1_bc, cc1, d1loc):
    """h1 = x@W1 per local node tile; s1/d1; write rec rows + local d table."""
    H, C, HC = m.heads, m.hid, m.heads * m.hid
    AL = mybir.AluOpType
    AF = mybir.ActivationFunctionType
    with tc.tile_pool(name="p0", bufs=2) as sb, \
         tc.tile_pool(name="p0c", bufs=1) as sbc, \
         tc.tile_pool(name="p0ps", bufs=2, space="PSUM") as ps:
        W1t = sbc.tile([HC, HC], bf16)
        nc.sync.dma_start(out=W1t[:], in_=W1b[:, :])
        a1t = sbc.tile([P, 2 * HC], bf16)
        nc.sync.dma_start(out=a1t[:], in_=a1_bc[:, :])
        for t in range(m.NT):
            xT = sb.tile([HC, P], bf16)
            nc.sync.dma_start_transpose(out=xT[:], in_=x_sl[t * P:(t + 1) * P, :])
            h1p = ps.tile([P, HC], f32, tag="h1p")
            nc.tensor.matmul(out=h1p[:], lhsT=xT[:], rhs=W1t[:], start=True, stop=True)
            rec = sb.tile([P, 2 * HC], bf16, tag="rec")
            nc.vector.memset(rec[:, HC + 2 * H:], 0.0)
            nc.scalar.activation(out=rec[:, 0:HC], in_=h1p[:], func=AF.Copy)
            # s1/d1: per-head reduce of h1*a
            prod = sb.tile([P, 2 * HC], bf16, tag="prod")
            nc.vector.tensor_tensor(
                out=prod[:].rearrange("p (k f) -> p k f", k=2),
                in0=rec[:, 0:HC].unsqueeze(1).to_broadcast([P, 2, HC]),
                in1=a1t[:].rearrange("p (k f) -> p k f", k=2), op=AL.mult)
            sd = sb.tile([P, 2 * H], f32, tag="sd")
            nc.vector.tensor_reduce(
                out=sd[:], in_=prod[:].rearrange("p (k h c) -> p (k h) c", k=2, h=H),
                axis=mybir.AxisListType.X, op=AL.add)
            # s1 (f32) into rec cols [HC : HC+2H(bf16)] as raw f32 bits
            nc.vector.tensor_copy(
                out=rec[:, HC:HC + 2 * H].bitcast(f32), in_=sd[:, 0:H])
            nc.sync.dma_start(out=cc1[t * P:(t + 1) * P, :], in_=rec[:])
            d1 = sb.tile([P, 4], f32, tag="d1")
            nc.vector.tensor_copy(out=d1[:, 0:H], in_=sd[:, H:2 * H])
            nc.sync.dma_start(out=d1loc[t * P:(t + 1) * P, 0:4], in_=d1[:])
        dum = sbc.tile([1, 4], f32)
        nc.vector.memset(dum[:], -300.0)
        nc.sync.dma_start(out=d1loc[m.NPC_pad + 0:m.NPC_pad + 1, 0:4], in_=dum[:])


def _gat_layer(nc, tc, m, layer, table, dloc, rec_elem, s_off, nh, ch, b_bc,
               iota_bc, rec_idx, d_idx, li_in, out_dram):
    """Edge-gather + one-hot segment softmax-sum for one GAT layer."""
    AL = mybir.AluOpType
    AF = mybir.ActivationFunctionType
    hcols = nh * ch                   # payload cols (128 or 32)
    rcols = hcols + nh                # rhs cols: [w*h | w]
    tstep = table.shape[1]            # row stride in elems
    # input column offsets
    rec_col_off = np.cumsum([0] + m.rec_cols)
    d_col_off = np.cumsum([0] + m.d_cols)
    li_col_off = np.cumsum([0] + m.li_cols)

    with tc.tile_pool(name=f"L{layer}", bufs=2) as sb, \
         tc.tile_pool(name=f"L{layer}c", bufs=1) as sbc, \
         tc.tile_pool(name=f"L{layer}g", bufs=8) as sg, \
         tc.tile_pool(name=f"L{layer}ps", bufs=3, space="PSUM") as ps:
        iota = sbc.tile([P, P], bf16)
        nc.sync.dma_start(out=iota[:], in_=iota_bc[:, :])
        bt = sbc.tile([P, hcols], f32)
        nc.sync.dma_start(out=bt[:], in_=b_bc[:, 0:hcols])
        for b in range(m.NB):
            B = m.batches[b]
            Tb = B["Tb"]
            if Tb == 0:
                continue
            # ---- load idx/li slices
            li = sb.tile([P, Tb], f32, tag="li")
            nc.sync.dma_start(out=li[:], in_=li_in[:, li_col_off[b]:li_col_off[b] + Tb])
            dxc = m.d_cols[b]
            dxt = sb.tile([P, dxc], i16, tag="dxt")
            nc.sync.dma_start(out=dxt[:], in_=d_idx[:, d_col_off[b]:d_col_off[b] + dxc])
            # ---- gathers
            CH = 8  # tiles per gather chunk (1024-index HW limit)
            rec = sb.tile([P, Tb, rec_elem], bf16, tag="rec")
            for q in range(4):
                Rq = B["Rq"][q]
                if Rq == 0:
                    continue
                ci = rec_col_off[4 * b + q]
                cn = m.rec_cols[4 * b + q]
                rxt = sb.tile([P, cn], i16, tag=f"rxt{q}")
                nc.sync.dma_start(out=rxt[:], in_=rec_idx[:, ci:ci + cn])
                r0 = B["reg_base"][q]
                lim = min(m.QN + 256, table.shape[0] - q * m.QN)
                for c0 in range(0, Rq, CH):
                    cT = min(CH, Rq - c0)
                    qn = nc._swq % 4
                    nc._swq += 1
                    _dma_gather_raw(
                        nc.gpsimd,
                        out_ap=rec[:, r0 + c0:r0 + c0 + cT, :],
                        in_ap=table[q * m.QN:q * m.QN + lim, 0:rec_elem],
                        idxs_ap=rxt[:, c0 * 8:(c0 + cT) * 8],
                        num_idxs=cT * P, elem_size=rec_elem, elem_step=tstep,
                        queue_num=qn)
            dg = sb.tile([P, Tb, nh], f32, tag="dg")
            for c0 in range(0, Tb, CH):
                cT = min(CH, Tb - c0)
                qn = nc._swq % 4
                nc._swq += 1
                _dma_gather_raw(
                    nc.gpsimd,
                    out_ap=dg[:, c0:c0 + cT, :],
                    in_ap=dloc[0:m.NPC_pad + 16, 0:nh],
                    idxs_ap=dxt[:, c0 * 8:(c0 + cT) * 8],
                    num_idxs=cT * P, elem_size=nh, elem_step=64,
                    queue_num=qn)
            # ---- attention weights (whole batch; contiguous tiles)
            s_ap = rec[:, :, s_off:s_off + 2 * nh].bitcast(f32)   # [P, Tb, nh]
            t4 = sb.tile([P, Tb, nh], f32, tag="t4")
            nc.vector.tensor_tensor(out=t4[:], in0=s_ap, in1=dg[:], op=AL.add)
            u4 = sb.tile([P, Tb, nh], f32, tag="u4")
            nc.vector.tensor_scalar_mul(u4[:], t4[:], NEG_SLOPE)
            nc.vector.tensor_tensor(out=t4[:], in0=t4[:], in1=u4[:], op=AL.max)
            w4 = sb.tile([P, Tb, nh], f32, tag="w4")
            nc.scalar.activation(out=w4[:], in_=t4[:], func=AF.Exp)
            # ---- per-window matmul accumulation (flat per-tile DVE ops)
            for w in B["ws"]:
                nw = int(m.Twq[w, :].sum())
                if nw == 0:
                    continue
                pw = ps.tile([P, rcols], f32, tag="pw")
                seen = 0
                for q in range(4):
                    Tq = int(m.Twq[w, q])
                    if Tq == 0:
                        continue
                    t0 = B["blk"][(w, q)]
                    for j in range(t0, t0 + Tq):
                        o = sg.tile([P, P], bf16, tag="og")
                        if nh == 1:
                            # weighted one-hot: O' = (iota==li) * w
                            nc.vector.tensor_scalar(
                                out=o[:], in0=iota[:],
                                scalar1=li[:, j:j + 1],
                                scalar2=w4[:, j, 0:1],
                                op0=AL.is_equal, op1=AL.mult)
                            rhs = rec[:, j, 0:rcols]
                        else:
                            nc.vector.tensor_scalar(
                                out=o[:], in0=iota[:],
                                scalar1=li[:, j:j + 1], scalar2=None,
                                op0=AL.is_equal)
                            r = sg.tile([P, rcols], bf16, tag="rg")
                            for h in range(nh):
                                # split head-muls across Vector and Scalar
                                if h % 2 == 0:
                                    nc.vector.tensor_scalar_mul(
                                        r[:, h * ch:(h + 1) * ch],
                                        rec[:, j, h * ch:(h + 1) * ch],
                                        w4[:, j, h:h + 1])
                                else:
                                    nc.scalar.mul(
                                        r[:, h * ch:(h + 1) * ch],
                                        rec[:, j, h * ch:(h + 1) * ch],
                                        w4[:, j, h:h + 1])
                            nc.scalar.copy(
                                out=r[:, hcols:rcols], in_=w4[:, j, :])
                            rhs = r[:]
                        nc.tensor.matmul(
                            out=pw[:], lhsT=o[:], rhs=rhs,
                            start=(seen == 0), stop=(seen == nw - 1))
                        seen += 1
                # ---- window epilogue: alpha-normalize + bias + ELU
                rcp = sb.tile([P, nh], f32, tag="rcp")
                nc.vector.reciprocal(rcp[:], pw[:, hcols:rcols])
                y0 = sb.tile([P, hcols], f32, tag="y0")
                nc.scalar.activation(out=y0[:], in_=pw[:, 0:hcols], func=AF.Copy)
                y = sb.tile([P, hcols], f32, tag="y")
                for h in range(nh):
                    nc.vector.tensor_scalar_mul(
                        y[:, h * ch:(h + 1) * ch], y0[:, h * ch:(h + 1) * ch],
                        rcp[:, h:h + 1])
                nc.vector.tensor_tensor(out=y[:], in0=y[:], in1=bt[:], op=AL.add)
                mn = sb.tile([P, hcols], f32, tag="mn")
                nc.vector.tensor_scalar_min(mn[:], y[:], 0.0)
                ex = sb.tile([P, hcols], f32, tag="ex")
                nc.scalar.activation(out=ex[:], in_=mn[:], func=AF.Exp)
                nc.vector.tensor_scalar_add(ex[:], ex[:], -1.0)
                nc.vector.tensor_scalar_max(y[:], y[:], 0.0)
                hf = sb.tile([P, hcols], bf16, tag="hf")
                nc.vector.tensor_tensor(out=hf[:], in0=y[:], in1=ex[:], op=AL.add)
                rows = min(P, m.NPC - w * P)
                nc.sync.dma_start(
                    out=out_dram[w * P:w * P + rows, :], in_=hf[0:rows, :])


def _phase2(nc, tc, m, h1x, W2b, a2_bc, cc2, d2loc):
    """h2 = h1x @ W2 (pre-activation), s2/d2; rec2 rows + local d2 table."""
    C, HC = m.hid, m.heads * m.hid
    AL = mybir.AluOpType
    AF = mybir.ActivationFunctionType
    with tc.tile_pool(name="p2", bufs=2) as sb, \
         tc.tile_pool(name="p2c", bufs=1) as sbc, \
         tc.tile_pool(name="p2ps", bufs=2, space="PSUM") as ps:
        # zero the h1x padding rows so transposed loads stay finite
        if m.NPC_pad > m.NPC:
            z = sbc.tile([P, HC], bf16)
            nc.vector.memset(z[:], 0.0)
            nc.sync.dma_start(out=h1x[m.NPC:m.NPC_pad, :],
                              in_=z[0:m.NPC_pad - m.NPC, :])
        W2t = sbc.tile([HC, C], bf16)
        nc.sync.dma_start(out=W2t[:], in_=W2b[:, :])
        a2t = sbc.tile([P, 2 * C], bf16)
        nc.sync.dma_start(out=a2t[:], in_=a2_bc[:, :])
        for t in range(m.NT):
            hT = sb.tile([HC, P], bf16)
            nc.sync.dma_start_transpose(out=hT[:], in_=h1x[t * P:(t + 1) * P, :])
            h2p = ps.tile([P, C], f32, tag="h2p")
            nc.tensor.matmul(out=h2p[:], lhsT=hT[:], rhs=W2t[:], start=True, stop=True)
            rec = sb.tile([P, P], bf16, tag="rec2")
            nc.vector.memset(rec[:, C + 4:], 0.0)
            nc.vector.memset(rec[:, C:C + 2], 1.0)  # ones col (+pad col)
            nc.scalar.activation(out=rec[:, 0:C], in_=h2p[:], func=AF.Copy)
            prod = sb.tile([P, 2 * C], bf16, tag="prod2")
            nc.vector.tensor_tensor(
                out=prod[:].rearrange("p (k f) -> p k f", k=2),
                in0=rec[:, 0:C].unsqueeze(1).to_broadcast([P, 2, C]),
                in1=a2t[:].rearrange("p (k f) -> p k f", k=2), op=AL.mult)
            sd = sb.tile([P, 2], f32, tag="sd2")
            nc.vector.tensor_reduce(
                out=sd[:], in_=prod[:].rearrange("p (k c) -> p k c", k=2),
                axis=mybir.AxisListType.X, op=AL.add)
            nc.vector.tensor_copy(out=rec[:, C + 2:C + 4].bitcast(f32),
                                  in_=sd[:, 0:1])
            nc.sync.dma_start(out=cc2[t * P:(t + 1) * P, :], in_=rec[:])
            d2 = sb.tile([P, 1], f32, tag="d2")
            nc.vector.tensor_copy(out=d2[:], in_=sd[:, 1:2])
            nc.sync.dma_start(out=d2loc[t * P:(t + 1) * P, 0:1], in_=d2[:])
        dum = sbc.tile([1, 1], f32)
        nc.vector.memset(dum[:], -300.0)
        nc.sync.dma_start(out=d2loc[m.NPC_pad:m.NPC_pad + 1, 0:1], in_=dum[:])


def _pool_final(nc, tc, m, hfin, gidA_in, gidB_in, iota_bc, recip_in,
                Wlin, blin, po_in, po_out, out_t, rg):
    AL = mybir.AluOpType
    AF = mybir.ActivationFunctionType
    C = m.hid
    with tc.tile_pool(name="pf", bufs=2) as sb, \
         tc.tile_pool(name="pfc", bufs=1) as sbc, \
         tc.tile_pool(name="pfps", bufs=1, space="PSUM") as ps:
        # zero pad rows of hfin
        if m.NPC_pad > m.NPC:
            z = sbc.tile([P, C], bf16)
            nc.vector.memset(z[:], 0.0)
            nc.sync.dma_start(out=hfin[m.NPC:m.NPC_pad, :],
                              in_=z[0:m.NPC_pad - m.NPC, :])
        iota = sbc.tile([P, P], bf16)
        nc.sync.dma_start(out=iota[:], in_=iota_bc[:, :])
        gA = sbc.tile([P, m.NT], bf16)
        nc.sync.dma_start(out=gA[:], in_=gidA_in[:, :])
        gB = sbc.tile([P, m.NT], bf16)
        nc.sync.dma_start(out=gB[:], in_=gidB_in[:, :])
        pA = ps.tile([P, C], f32, tag="pA")
        pB = ps.tile([P, C], f32, tag="pB")
        for t in range(m.NT):
            h = sb.tile([P, C], bf16, tag="h")
            nc.sync.dma_start(out=h[:], in_=hfin[t * P:(t + 1) * P, :])
            for g_t, acc in ((gA, pA), (gB, pB)):
                O = sb.tile([P, P], bf16, tag="Opool")
                nc.vector.tensor_tensor(
                    out=O[:], in0=iota[:],
                    in1=g_t[:, t:t + 1].to_broadcast([P, P]), op=AL.is_equal)
                nc.tensor.matmul(out=acc[:], lhsT=O[:], rhs=h[:],
                                 start=(t == 0), stop=(t == m.NT - 1))
        sA = sb.tile([P, C], f32)
        nc.vector.tensor_copy(out=sA[:], in_=pA[:])
        sB = sb.tile([P, C], f32)
        nc.vector.tensor_copy(out=sB[:], in_=pB[:])
        nc.sync.dma_start(out=po_in[0:P, :], in_=sA[:])
        nc.sync.dma_start(out=po_in[P:256, :], in_=sB[:])
        nc.gpsimd.collective_compute(
            kind="AllReduce", op=AL.add, replica_groups=rg,
            ins=[po_in[:, :]], outs=[po_out[:, :]])
        # mean + final linear
        rcp = sbc.tile([P, 2], f32)
        nc.sync.dma_start(out=rcp[:], in_=recip_in[:, :])
        ident = sbc.tile([P, P], f32)
        make_identity(nc, ident[:])
        WT = sbc.tile([C, 10], f32)
        nc.sync.dma_start(out=WT[:], in_=Wlin[:, :])
        bl = sbc.tile([10, 1], f32)
        nc.sync.dma_start(out=bl[:], in_=blin[:, :])
        poT = sb.tile([C, 256], f32)
        for half in range(2):
            pm = sb.tile([P, C], f32, tag="pm")
            nc.sync.dma_start(out=pm[:], in_=po_out[half * P:(half + 1) * P, :])
            nc.vector.tensor_scalar(
                out=pm[:], in0=pm[:], scalar1=rcp[:, half:half + 1],
                scalar2=None, op0=AL.mult)
            tp = ps.tile([C, P], f32, tag="tp")
            nc.tensor.transpose(out=tp[:], in_=pm[:], identity=ident[:])
            nc.vector.tensor_copy(out=poT[:, half * P:(half + 1) * P], in_=tp[:])
        om = ps.tile([10, 256], f32, tag="om")
        nc.tensor.matmul(out=om[:], lhsT=WT[:], rhs=poT[:], start=True, stop=True)
        ob = sb.tile([10, 256], f32)
        nc.scalar.activation(out=ob[:], in_=om[:], func=AF.Identity, bias=bl[:, 0:1])
        for half in range(2):
            tp2 = ps.tile([P, 10], f32, tag="tp2")
            nc.tensor.transpose(out=tp2[:], in_=ob[:, half * P:(half + 1) * P],
                                identity=ident[0:10, 0:10])
            oo = sb.tile([P, 10], f32, tag="oo")
            nc.vector.tensor_copy(out=oo[:], in_=tp2[:])
            nc.sync.dma_start(out=out_t[half * P:(half + 1) * P, :], in_=oo[:])


# ---------------------------------------------------------------- entry point

def kernel(x, edge_index, batch, W1, a_src1, a_dst1, b1, W2, a_src2, a_dst2,
           b2, W_lin, b_lin):
    global _last_exec_ns
    x = np.asarray(x)
    N, IN_C = x.shape
    heads, hid = np.asarray(a_src1).shape
    m = _host_prep(x, np.asarray(edge_index), np.asarray(batch), heads, hid)

    nc = _build(m)

    bfl = ml_dtypes.bfloat16
    HC = heads * hid
    in_maps = []
    a1 = np.concatenate([np.asarray(a_src1).reshape(-1),
                         np.asarray(a_dst1).reshape(-1)]).astype(bfl)
    a2 = np.concatenate([np.asarray(a_src2).reshape(-1),
                         np.asarray(a_dst2).reshape(-1)]).astype(bfl)
    iota = np.tile(np.arange(P, dtype=np.float64), (P, 1)).astype(bfl)
    recip2 = np.stack([m.recip[0:P], m.recip[P:256]], 1).astype(np.float32)
    for c in range(NCORES):
        pc = m.per_core[c]
        xs = np.zeros((m.NPC_pad, HC), bfl)
        xs[0:m.NPC] = x[c * m.NPC:(c + 1) * m.NPC].astype(bfl)
        in_maps.append({
            "x_sl": xs,
            "W1b": np.asarray(W1).astype(bfl),
            "a1_bc": np.tile(a1, (P, 1)),
            "b1_bc": np.tile(np.asarray(b1).reshape(1, -1), (P, 1)).astype(np.float32),
            "W2b": np.asarray(W2).astype(bfl),
            "a2_bc": np.tile(a2, (P, 1)),
            "b2_bc": np.tile(np.asarray(b2).reshape(1, -1), (P, 1)).astype(np.float32),
            "Wlin": np.asarray(W_lin).astype(np.float32),
            "blin": np.asarray(b_lin).reshape(10, 1).astype(np.float32),
            "recip_in": recip2,
            "iota_bc": iota,
            "rec_idx": pc["rec_idx"],
            "d_idx": pc["d_idx"],
            "li_in": pc["li"],
            "gidA": pc["gidA"],
            "gidB": pc["gidB"],
        })

    import os
    if os.environ.get("GAT_SIM"):
        from concourse.bass_interp import MultiCoreSim
        mcs = MultiCoreSim(nc, NCORES, require_finite=False, require_nnan=False)
        for c in range(NCORES):
            core = mcs.cores[c]
            for k, v in in_maps[c].items():
                core.tensor(k)[:] = v
        mcs.simulate()
        return np.ascontiguousarray(np.asarray(mcs.cores[0].mem_tensor("out")))

    want_trace = bool(os.environ.get("GAT_TRACE"))
    if want_trace:
        _install_ntff_hook()
    try:
        res = run_bass_kernel_spmd(nc, in_maps, core_ids=list(range(NCORES)),
                                   trace=want_trace)
    except ModuleNotFoundError:
        res = run_bass_kernel_spmd(nc, in_maps, core_ids=list(range(NCORES)),
                                   trace=False)
    _last_exec_ns = res.exec_time_ns
    return np.ascontiguousarray(res.results[0]["out"])


def run(x, edge_index, batch, W1, a_src1, a_dst1, b1, W2, a_src2, a_dst2,
        b2, W_lin, b_lin):
    return kernel(x, edge_index, batch, W1, a_src1, a_dst1, b1, W2, a_src2,
                  a_dst2, b2, W_lin, b_lin)



# revision 10
# speedup vs baseline: 1.7155x; 1.7155x over previous
"""GAT (2-layer, 4/1 heads) on 8 trn2 NeuronCores via Bass/Tile.

Strategy (dst-partitioned, gather-based):
- Edges (+self loops) sorted by dst; each core owns a contiguous dst range
  (N/8 nodes). Per 128-dst window, the segment softmax+sum is computed via
  one-hot matmuls accumulating in PSUM.
- Per-node records (h interleaved with ones + attention s-values) live in a
  DRAM table with 256B-multiple row stride; per-edge rows are fetched with
  InstDMAGatherAnt (int16 indices relative to a src-quarter base). Per-edge
  d-values come from a per-core-local d table (dst-local indices, dummy row
  for padding).
- Node tables are built per-core then AllGather'd in chunks (chunk-major row
  layout) so the collective overlaps the producing compute loop.
- phase2 (h1 @ W2 + s2/d2) is inlined per-window into the L1 loop via PE
  transpose; graph pooling is inlined per-window into the L2 loop.
- Final: AllReduce pooled sums, mean, tiny linear.
"""

import math

import numpy as np
import ml_dtypes

import concourse.bass as bass
import concourse.mybir as mybir
import concourse.tile as tile
from concourse import bacc
from concourse.bass_utils import run_bass_kernel_spmd
from concourse.masks import make_identity

NCORES = 8
P = 128
NEG_SLOPE = 0.2
BWIN = 6          # windows per batch
NCH = 7           # AllGather chunks (must divide NT)
CH = 8            # tiles per gather chunk (1024-index HW limit)

bf16 = mybir.dt.bfloat16
f32 = mybir.dt.float32
i16 = mybir.dt.int16

_last_exec_ns = None


def _install_ntff_hook():
    """Provide antenv.axon_hooks (missing on this image) so trace=True works."""
    import sys
    import types
    try:
        from antenv import axon_hooks  # noqa: F401
        return
    except ImportError:
        pass
    import antenv
    mod = types.ModuleType("antenv.axon_hooks")
    mod._hook = None
    mod.set_axon_ntff_profile_hook = lambda h: setattr(mod, "_hook", h)
    mod.get_axon_ntff_profile_hook = lambda: mod._hook
    sys.modules["antenv.axon_hooks"] = mod
    antenv.axon_hooks = mod
    try:
        from trn_agent_boot.trn_boot import _ntff_profile_via_ctypes
        mod._hook = _ntff_profile_via_ctypes("/opt/axon/libaxon_pjrt.so")
    except Exception:
        mod._hook = None
    # avoid remote artifact uploads in the trace path (sandbox is zero-egress)
    import concourse.bass_utils as bu
    bu.upload_artifacts = lambda tmpdir: f"local:{tmpdir}"


# ---------------------------------------------------------------- host helpers

def _wrap16(flat, pad_val=0):
    """int16 index list -> [128, ceil(n/16)] wrapped+replicated layout."""
    n = len(flat)
    cols = (n + 15) // 16
    a = np.full(cols * 16, pad_val, np.int16)
    a[:n] = flat
    w = a.reshape(cols, 16).T  # [16, cols]
    return np.tile(w, (8, 1))  # [128, cols]


def _slotmajor(flat, T, dtype):
    """slot-stream [T*128] -> [128, T] (slot i -> partition i%128, block i//128)."""
    return np.ascontiguousarray(flat.reshape(T, P).T.astype(dtype))


class Meta:
    pass


def _host_prep(x, edge_index, batch, heads, hid):
    N = x.shape[0]
    assert N % NCORES == 0
    NPC = N // NCORES
    NT = (NPC + P - 1) // P          # node tiles / windows per core
    NPC_pad = NT * P
    nch = next(k for k in (NCH, 6, 5, 4, 3, 2, 1) if NT % k == 0)
    CR = NPC_pad // nch              # chunk rows per core
    TROWS = NPC_pad * NCORES         # shared table rows (chunk-major layout)
    QN = TROWS // 4                  # src quarter size (in permuted row space)
    assert QN + 256 < 32768, "quarter too big for int16 gather indices"
    assert NPC_pad + 16 < 32768

    # global node id -> permuted (chunk-major) table row
    def perm_row(n):
        c = n // NPC
        loc = n - c * NPC
        k = loc // CR
        return k * (NCORES * CR) + c * CR + (loc - k * CR)

    E0 = edge_index.shape[1]
    src = np.concatenate([np.asarray(edge_index[0]), np.arange(N)]).astype(np.int64)
    dst = np.concatenate([np.asarray(edge_index[1]), np.arange(N)]).astype(np.int64)
    order = np.argsort(dst, kind="stable")
    src, dst = src[order], dst[order]
    # vectorized permuted row for all srcs
    sc = src // NPC
    sloc = src - sc * NPC
    sk = sloc // CR
    srow = sk * (NCORES * CR) + sc * CR + (sloc - sk * CR)

    core_edges = []
    for c in range(NCORES):
        lo = np.searchsorted(dst, c * NPC, "left")
        hi = np.searchsorted(dst, (c + 1) * NPC, "left")
        core_edges.append((srow[lo:hi], dst[lo:hi]))

    NB = (NT + BWIN - 1) // BWIN     # batches
    # per (core, window, quarter) edge lists (srow = permuted table row)
    cell = [[[None] * 4 for _ in range(NT)] for _ in range(NCORES)]
    for c in range(NCORES):
        s_c, d_c = core_edges[c]
        w_of = (d_c - c * NPC) // P
        q_of = s_c // QN
        for w in range(NT):
            m = w_of == w
            sw, dw, qw = s_c[m], d_c[m], q_of[m]
            for q in range(4):
                mq = qw == q
                swq, dwq = sw[mq], dw[mq]
                # sort by (permuted) src row for HBM gather locality
                o2 = np.argsort(swq, kind="stable")
                cell[c][w][q] = (swq[o2], dwq[o2])

    # equalized tile counts per (window, quarter)
    Twq = np.zeros((NT, 4), np.int64)
    for w in range(NT):
        for q in range(4):
            mx = max(len(cell[c][w][q][0]) for c in range(NCORES))
            Twq[w, q] = (mx + P - 1) // P

    m = Meta()
    m.N, m.NPC, m.NT, m.NPC_pad, m.QN, m.NB = N, NPC, NT, NPC_pad, QN, NB
    m.CR, m.TROWS, m.NCH = CR, TROWS, nch
    m.heads, m.hid = heads, hid
    m.Twq = Twq
    # per-batch structure
    m.batches = []
    for b in range(NB):
        ws = list(range(b * BWIN, min((b + 1) * BWIN, NT)))
        Rq = [int(Twq[ws, q].sum()) for q in range(4)]
        Tb = sum(Rq)
        reg_base = np.cumsum([0] + Rq)[:4]
        blk = {}
        for q in range(4):
            off = reg_base[q]
            for w in ws:
                blk[(w, q)] = int(off)
                off += int(Twq[w, q])
        m.batches.append(dict(ws=ws, Rq=Rq, Tb=Tb, blk=blk, reg_base=reg_base))

    # per-core input arrays
    m.rec_cols = []   # per (b,q) col counts in rec_idx array
    per_core = []
    for c in range(NCORES):
        rec_idx_cols = []
        d_idx_cols = []
        li_cols = []
        for b in range(NB):
            B = m.batches[b]
            d_flat = np.zeros(B["Tb"] * P, np.int64)
            li_flat = np.zeros(B["Tb"] * P, np.int64)
            for q in range(4):
                r_flat = np.zeros(B["Rq"][q] * P, np.int64)
                for w in B["ws"]:
                    sw, dw = cell[c][w][q]
                    t0 = B["blk"][(w, q)]
                    nsl = int(Twq[w, q]) * P
                    sl = slice(t0 * P, t0 * P + nsl)
                    # pad: src->quarter base row 0 (w==0 via dummy d row)
                    rr = np.zeros(nsl, np.int64)
                    dd = np.full(nsl, NPC_pad, np.int64)  # dummy d row (-300)
                    ll = np.zeros(nsl, np.int64)
                    k = len(sw)
                    rr[:k] = sw - q * QN
                    dd[:k] = dw - c * NPC
                    ll[:k] = dw - (c * NPC + w * P)
                    r_flat[t0 * P - B["reg_base"][q] * P:
                           t0 * P - B["reg_base"][q] * P + nsl] = rr
                    d_flat[sl] = dd
                    li_flat[sl] = ll
                rec_idx_cols.append(_wrap16(r_flat.astype(np.int16)))
                if c == 0:
                    m.rec_cols.append(rec_idx_cols[-1].shape[1])
            d_idx_cols.append(_wrap16(d_flat.astype(np.int16)))
            li_cols.append(_slotmajor(li_flat, B["Tb"], np.float32))
        pc = dict(
            rec_idx=np.concatenate(rec_idx_cols, 1) if rec_idx_cols else
            np.zeros((P, 0), np.int16),
            d_idx=np.concatenate(d_idx_cols, 1),
            li=np.concatenate(li_cols, 1),
        )
        per_core.append(pc)
    m.d_cols = []
    m.li_cols = []
    for b in range(NB):
        m.d_cols.append((m.batches[b]["Tb"] * P + 15) // 16)
        m.li_cols.append(m.batches[b]["Tb"])

    # graph pooling metadata
    G = int(np.max(batch)) + 1
    m.G = G
    assert G <= 256
    counts = np.bincount(np.asarray(batch).astype(np.int64), minlength=256)
    recip = (1.0 / np.maximum(counts, 1)).astype(np.float32)
    m.recip = recip  # [256]
    for c in range(NCORES):
        gid = np.full(NPC_pad, -1, np.int64)
        gid[:NPC] = np.asarray(batch)[c * NPC:(c + 1) * NPC]
        gA = gid.astype(np.float64)
        gB = np.where(gid >= 0, gid - 128, -1).astype(np.float64)
        per_core[c]["gidA"] = _slotmajor(gA, NT, ml_dtypes.bfloat16)
        per_core[c]["gidB"] = _slotmajor(gB, NT, ml_dtypes.bfloat16)
    m.per_core = per_core
    return m


# ---------------------------------------------------------------- raw dma_gather

def _dma_gather_raw(gp, out_ap, in_ap, idxs_ap, num_idxs, elem_size, elem_step,
                    queue_num=0):
    """dma_gather without the elem%256B assert (stride must be 256B-mult)."""
    from concourse import ap_utils
    from concourse._compat import exact_div
    assert idxs_ap.dtype == i16
    assert in_ap.dtype == out_ap.dtype
    assert ap_utils.ap_is_contiguous(in_ap.ap[1:])
    assert ap_utils.ap_is_contiguous(out_ap.ap[1:])
    assert ap_utils.ap_is_contiguous(idxs_ap.ap[1:])
    assert in_ap.ap[0][0] == elem_step
    stride_bytes = elem_step * mybir.dt.size(in_ap.dtype)
    stride_256 = exact_div(stride_bytes, 256)
    assert stride_256 < 256
    _in_ap = gp.lower_ap_dma(in_ap, for_custom_bir_dma=True)
    _idxs_ap = gp.lower_ap(idxs_ap)
    _out_ap = gp.lower_ap(out_ap)
    return gp.add_instruction(
        mybir.InstDMAGatherAnt(
            name=gp.bass.get_next_instruction_name(),
            ins=[*_in_ap, _idxs_ap, gp.lower_val_access(gp.to_reg(num_idxs))],
            outs=[_out_ap],
            transpose=False,
            num_idxs=num_idxs,
            elem_size=elem_size,
            stride_bytes_256=stride_256,
            gen_mode=0,
            single_packet=True,
            queue_num=queue_num,
            sbuf_tokens_per_rank=0,
            sbuf_free_dim_per_rank=0,
            sbuf_free_dim_pad_per_rank=0,
            sbuf_byte_offset=0,
        )
    )


# ---------------------------------------------------------------- device program

def _build(m):
    nc = bacc.Bacc("TRN2", target_bir_lowering=False, debug=False,
                   num_devices=NCORES, num_swdge_queues=4)
    nc._swq = 0
    H, C = m.heads, m.hid
    HC = H * C                       # 128
    NPC_pad, NT, NB, QN = m.NPC_pad, m.NT, m.NB, m.QN
    R1 = H * (C + 1) + 2 * H         # rec1 elem: 4x[h(32)|1] + s_f32(8 bf16)
    R2 = C + 2 + 2                   # rec2 elem: [h2(32)|1|pad] + s2_f32(2 bf16)

    # ---------------- inputs
    def ein(name, shape, dt):
        return nc.dram_tensor(name, shape, dt, kind="ExternalInput")

    xT_sl = ein("xT_sl", [HC, NPC_pad], bf16)    # pre-transposed x slice
    W1b = ein("W1b", [HC, HC], bf16)
    a1_bc = ein("a1_bc", [P, 2 * HC], bf16)      # [asrc1(128) | adst1(128)] rows replicated
    b1_bc = ein("b1_bc", [P, HC], f32)
    W2b = ein("W2b", [HC, C], bf16)
    a2_bc = ein("a2_bc", [P, 2 * C], bf16)
    b2_bc = ein("b2_bc", [P, C], f32)
    Wlin = ein("Wlin", [C, 10], f32)
    blin = ein("blin", [10, 1], f32)
    recip_in = ein("recip_in", [P, 2], f32)
    iota_bc = ein("iota_bc", [P, P], bf16)
    rec_idx = ein("rec_idx", [P, sum(m.rec_cols)], i16)
    d_idx = ein("d_idx", [P, sum(m.d_cols)], i16)
    li_in = ein("li_in", [P, sum(m.li_cols)], f32)
    gidA_in = ein("gidA", [P, NT], bf16)
    gidB_in = ein("gidB", [P, NT], bf16)

    out_t = nc.dram_tensor("out", [256, 10], f32, kind="ExternalOutput")

    # ---------------- internal DRAM
    cc1 = nc.dram_tensor("cc1", [NPC_pad, 2 * HC], bf16, kind="Internal")
    table1 = nc.dram_tensor("table1", [m.TROWS + P, 2 * HC], bf16, kind="Internal",
                            addr_space="Shared")
    d1loc = nc.dram_tensor("d1loc", [NPC_pad + 16, 64], f32, kind="Internal")
    cc2 = nc.dram_tensor("cc2", [NPC_pad, P], bf16, kind="Internal")
    table2 = nc.dram_tensor("table2", [m.TROWS + P, P], bf16, kind="Internal",
                            addr_space="Shared")
    d2loc = nc.dram_tensor("d2loc", [NPC_pad + 16, 64], f32, kind="Internal")
    po_in = nc.dram_tensor("po_in", [256, C], f32, kind="Internal")
    po_out = nc.dram_tensor("po_out", [256, C], f32, kind="Internal")

    AL = mybir.AluOpType
    rg = [list(range(NCORES))]
    CRk = m.CR

    with tile.TileContext(nc) as tc:
        _phase0(nc, tc, m, xT_sl, W1b, a1_bc, cc1, d1loc, table1, rg)
        _gat_layer1(nc, tc, m, table1, d1loc, b1_bc, iota_bc, rec_idx, d_idx,
                    li_in, W2b, a2_bc, cc2, d2loc, table2, rg)
        _gat_layer2(nc, tc, m, table2, d2loc, b2_bc, iota_bc, rec_idx, d_idx,
                    li_in, gidA_in, gidB_in, recip_in, Wlin, blin,
                    po_in, po_out, out_t, rg)

    nc.compile()
    return nc


def _phase0(nc, tc, m, xT_sl, W1b, a1_bc, cc1, d1loc, table1, rg):
    """h1 = x@W1 per local node tile; s1/d1; rec rows + local d table.

    rec1 layout (interleaved): [h0(32)|1|h1(32)|1|h2(32)|1|h3(32)|1|s1(4xf32)]
    AllGather'd to table1 in NCH chunks as windows complete.
    """
    H, C, HC = m.heads, m.hid, m.heads * m.hid
    AL = mybir.AluOpType
    AF = mybir.ActivationFunctionType
    WPCH = m.NT // m.NCH             # windows per AllGather chunk
    with tc.tile_pool(name="p0", bufs=3) as sb, \
         tc.tile_pool(name="p0c", bufs=1) as sbc, \
         tc.tile_pool(name="p0ps", bufs=3, space="PSUM") as ps:
        xT = sbc.tile([HC, m.NPC_pad], bf16)
        nc.sync.dma_start(out=xT[:], in_=xT_sl[:, :])
        W1t = sbc.tile([HC, HC], bf16)
        nc.sync.dma_start(out=W1t[:], in_=W1b[:, :])
        a1t = sbc.tile([P, 2 * HC], bf16)
        nc.sync.dma_start(out=a1t[:], in_=a1_bc[:, :])
        for t in range(m.NT):
            h1p = ps.tile([P, HC], f32, tag="h1p")
            nc.tensor.matmul(out=h1p[:], lhsT=xT[:, t * P:(t + 1) * P],
                             rhs=W1t[:], start=True, stop=True)
            rec = sb.tile([P, 2 * HC], bf16, tag="rec")
            # ones cols at 33k+32, zero tail after s
            nc.vector.memset(rec[:, 4 * (C + 1):], 0.0)
            on = rec[:, 0:4 * (C + 1)].rearrange("p (h c) -> p h c", h=H)
            nc.vector.memset(on[:, :, C:C + 1], 1.0)
            h_v = on[:, :, 0:C]                          # [P, 4, 32] strided
            nc.scalar.activation(
                out=h_v, in_=h1p[:].rearrange("p (h c) -> p h c", h=H),
                func=AF.Copy)
            # s1/d1: per-head reduce of h1*a
            prod = sb.tile([P, 2, H, C], bf16, tag="prod")
            nc.vector.tensor_tensor(
                out=prod[:],
                in0=h_v.unsqueeze(1).to_broadcast([P, 2, H, C]),
                in1=a1t[:].rearrange("p (k h c) -> p k h c", k=2, h=H),
                op=AL.mult)
            sd = sb.tile([P, 2 * H], f32, tag="sd")
            nc.vector.tensor_reduce(
                out=sd[:], in_=prod[:].rearrange("p k h c -> p (k h) c"),
                axis=mybir.AxisListType.X, op=AL.add)
            # s1 (f32) into rec cols [132 : 132+8(bf16)] as raw f32 bits
            nc.vector.tensor_copy(
                out=rec[:, 4 * (C + 1):4 * (C + 1) + 2 * H].bitcast(f32),
                in_=sd[:, 0:H])
            nc.sync.dma_start(out=cc1[t * P:(t + 1) * P, :], in_=rec[:])
            d1 = sb.tile([P, 4], f32, tag="d1")
            nc.vector.tensor_copy(out=d1[:, 0:H], in_=sd[:, H:2 * H])
            nc.sync.dma_start(out=d1loc[t * P:(t + 1) * P, 0:4], in_=d1[:])
            if (t + 1) % WPCH == 0:
                k = (t + 1) // WPCH - 1
                nc.gpsimd.collective_compute(
                    kind="AllGather", op=AL.bypass, replica_groups=rg,
                    ins=[cc1[k * m.CR:(k + 1) * m.CR, :]],
                    outs=[table1[k * m.CR * NCORES:(k + 1) * m.CR * NCORES, :]])
        dum = sbc.tile([1, 4], f32)
        nc.vector.memset(dum[:], -300.0)
        nc.sync.dma_start(out=d1loc[m.NPC_pad + 0:m.NPC_pad + 1, 0:4], in_=dum[:])


def _load_batch(nc, sb, m, b, table, dloc, rec_elem, nh, rec_idx, d_idx, li_in,
                rec_col_off, d_col_off, li_col_off):
    """Issue idx loads + gathers for batch b; returns (li, rec, dg) tiles."""
    B = m.batches[b]
    Tb = B["Tb"]
    tstep = table.shape[1]
    li = sb.tile([P, Tb], f32, tag="li")
    nc.sync.dma_start(out=li[:], in_=li_in[:, li_col_off[b]:li_col_off[b] + Tb])
    dxc = m.d_cols[b]
    dxt = sb.tile([P, dxc], i16, tag="dxt")
    nc.sync.dma_start(out=dxt[:], in_=d_idx[:, d_col_off[b]:d_col_off[b] + dxc])
    rec = sb.tile([P, Tb, rec_elem], bf16, tag="rec")
    for q in range(4):
        Rq = B["Rq"][q]
        if Rq == 0:
            continue
        ci = rec_col_off[4 * b + q]
        cn = m.rec_cols[4 * b + q]
        rxt = sb.tile([P, cn], i16, tag=f"rxt{q}")
        nc.sync.dma_start(out=rxt[:], in_=rec_idx[:, ci:ci + cn])
        r0 = B["reg_base"][q]
        lim = min(m.QN + 256, table.shape[0] - q * m.QN)
        for c0 in range(0, Rq, CH):
            cT = min(CH, Rq - c0)
            qn = nc._swq % 4
            nc._swq += 1
            _dma_gather_raw(
                nc.gpsimd,
                out_ap=rec[:, r0 + c0:r0 + c0 + cT, :],
                in_ap=table[q * m.QN:q * m.QN + lim, 0:rec_elem],
                idxs_ap=rxt[:, c0 * 8:(c0 + cT) * 8],
                num_idxs=cT * P, elem_size=rec_elem, elem_step=tstep,
                queue_num=qn)
    dg = sb.tile([P, Tb, nh], f32, tag="dg")
    for c0 in range(0, Tb, CH):
        cT = min(CH, Tb - c0)
        qn = nc._swq % 4
        nc._swq += 1
        _dma_gather_raw(
            nc.gpsimd,
            out_ap=dg[:, c0:c0 + cT, :],
            in_ap=dloc[0:m.NPC_pad + 16, 0:nh],
            idxs_ap=dxt[:, c0 * 8:(c0 + cT) * 8],
            num_idxs=cT * P, elem_size=nh, elem_step=64,
            queue_num=qn)
    return li, rec, dg


def _edge_weights(nc, sb, rec, dg, s_off, nh, Tb):
    """w4 = exp(leaky_relu(s_src + d_dst)) for the whole batch."""
    AL = mybir.AluOpType
    AF = mybir.ActivationFunctionType
    s_ap = rec[:, :, s_off:s_off + 2 * nh].bitcast(f32)   # [P, Tb, nh]
    t4 = sb.tile([P, Tb, nh], f32, tag="t4")
    nc.vector.tensor_tensor(out=t4[:], in0=s_ap, in1=dg[:], op=AL.add)
    u4 = sb.tile([P, Tb, nh], f32, tag="u4")
    nc.vector.tensor_scalar_mul(u4[:], t4[:], NEG_SLOPE)
    nc.vector.tensor_tensor(out=t4[:], in0=t4[:], in1=u4[:], op=AL.max)
    w4 = sb.tile([P, Tb, nh], f32, tag="w4")
    nc.scalar.activation(out=w4[:], in_=t4[:], func=AF.Exp)
    return w4


def _build_cell(nc, sg, iota, li, rec, w4, nh, rcols, j0, T):
    """Batched one-hot + weighted-rhs build for T consecutive tiles."""
    AL = mybir.AluOpType
    o = sg.tile([P, T, P], bf16, tag="og")
    nc.vector.tensor_tensor(
        out=o[:], in0=iota[:].unsqueeze(1).to_broadcast([P, T, P]),
        in1=li[:, j0:j0 + T].unsqueeze(2).to_broadcast([P, T, P]),
        op=AL.is_equal)
    r = sg.tile([P, T, rcols], bf16, tag="rg")
    if nh == 1:
        nc.vector.tensor_tensor(
            out=r[:], in0=rec[:, j0:j0 + T, 0:rcols],
            in1=w4[:, j0:j0 + T, :].to_broadcast([P, T, rcols]),
            op=AL.mult)
    else:
        cpo = rcols // nh
        nc.vector.tensor_tensor(
            out=r[:].rearrange("p t (h c) -> p t h c", h=nh),
            in0=rec[:, j0:j0 + T, 0:rcols].rearrange(
                "p t (h c) -> p t h c", h=nh),
            in1=w4[:, j0:j0 + T, :].unsqueeze(3).to_broadcast([P, T, nh, cpo]),
            op=AL.mult)
    return o, r


def _win_matmuls(nc, m, b, w, o_tiles, pw, col0, rcols):
    """Issue the one-hot matmuls of window w into pw[:, col0:col0+rcols]."""
    B = m.batches[b]
    nw = int(m.Twq[w, :].sum())
    seen = 0
    for q in range(4):
        Tq = int(m.Twq[w, q])
        if Tq == 0:
            continue
        t0 = B["blk"][(w, q)]
        for j in range(t0, t0 + Tq):
            o, r, jbase = o_tiles[j]
            nc.tensor.matmul(
                out=pw[:, col0:col0 + rcols],
                lhsT=o[:, j - jbase, :], rhs=r[:, j - jbase, :],
                start=(seen == 0), stop=(seen == nw - 1))
            seen += 1


def _alpha_elu(nc, sb, pw, bt, nh, ch, nwin, tag):
    """pw [P, nwin, nh*(ch+1)] interleaved -> hf [P, nwin, nh*ch] bf16."""
    AL = mybir.AluOpType
    AF = mybir.ActivationFunctionType
    hcols = nh * ch
    pv = pw[:].rearrange("p w (h c) -> p w h c", h=nh)    # c = ch+1
    rcp = sb.tile([P, nwin, nh], f32, tag=f"rcp{tag}")
    nc.vector.reciprocal(rcp[:], pv[:, :, :, ch:ch + 1].rearrange(
        "p w h c -> p w (h c)"))
    y = sb.tile([P, nwin, nh, ch], f32, tag=f"y{tag}")
    nc.vector.tensor_tensor(
        out=y[:], in0=pv[:, :, :, 0:ch],
        in1=rcp[:].unsqueeze(3).to_broadcast([P, nwin, nh, ch]), op=AL.mult)
    yf = y[:].rearrange("p w h c -> p w (h c)")
    nc.vector.tensor_tensor(
        out=yf, in0=yf,
        in1=bt[:].unsqueeze(1).to_broadcast([P, nwin, hcols]), op=AL.add)
    mn = sb.tile([P, nwin, hcols], f32, tag=f"mn{tag}")
    nc.vector.tensor_scalar_min(mn[:], yf, 0.0)
    ex = sb.tile([P, nwin, hcols], f32, tag=f"ex{tag}")
    nc.scalar.activation(out=ex[:], in_=mn[:], func=AF.Exp)
    nc.vector.tensor_scalar_add(ex[:], ex[:], -1.0)
    nc.vector.tensor_scalar_max(yf, yf, 0.0)
    hf = sb.tile([P, nwin, hcols], bf16, tag=f"hf{tag}")
    nc.vector.tensor_tensor(out=hf[:], in0=yf, in1=ex[:], op=AL.add)
    return hf


def _gat_layer1(nc, tc, m, table1, d1loc, b1_bc, iota_bc, rec_idx, d_idx,
                li_in, W2b, a2_bc, cc2, d2loc, table2, rg):
    """L1 edge loop + inlined phase2 (h1@W2, s2/d2, cc2 chunked AllGather)."""
    H, C = m.heads, m.hid
    HC = H * C
    AL = mybir.AluOpType
    AF = mybir.ActivationFunctionType
    R1 = H * (C + 1) + 2 * H
    rcols = H * (C + 1)              # 132
    s_off = rcols
    WPCH = m.NT // m.NCH
    rec_col_off = np.cumsum([0] + m.rec_cols)
    d_col_off = np.cumsum([0] + m.d_cols)
    li_col_off = np.cumsum([0] + m.li_cols)
    ag_done = 0

    with tc.tile_pool(name="L1", bufs=2) as sb, \
         tc.tile_pool(name="L1c", bufs=1) as sbc, \
         tc.tile_pool(name="L1g", bufs=6) as sg, \
         tc.tile_pool(name="L1e", bufs=2) as se, \
         tc.tile_pool(name="L1ps", bufs=2, space="PSUM") as ps, \
         tc.tile_pool(name="L1ps2", bufs=2, space="PSUM") as ps2:
        iota = sbc.tile([P, P], bf16)
        nc.sync.dma_start(out=iota[:], in_=iota_bc[:, :])
        bt = sbc.tile([P, HC], f32)
        nc.sync.dma_start(out=bt[:], in_=b1_bc[:, 0:HC])
        W2t = sbc.tile([HC, C], bf16)
        nc.sync.dma_start(out=W2t[:], in_=W2b[:, :])
        a2t = sbc.tile([P, 2 * C], bf16)
        nc.sync.dma_start(out=a2t[:], in_=a2_bc[:, :])
        ident = sbc.tile([P, P], bf16)
        make_identity(nc, ident[:])

        for b in range(m.NB):
            B = m.batches[b]
            Tb = B["Tb"]
            if Tb == 0:
                continue
            li, rec, dg = _load_batch(
                nc, sb, m, b, table1, d1loc, R1, H, rec_idx, d_idx, li_in,
                rec_col_off, d_col_off, li_col_off)
            w4 = _edge_weights(nc, sb, rec, dg, s_off, H, Tb)
            # build o/r per cell, in consumption (window-major) order
            o_tiles = {}
            for w in B["ws"]:
                for q in range(4):
                    Tq = int(m.Twq[w, q])
                    if Tq == 0:
                        continue
                    t0 = B["blk"][(w, q)]
                    o, r = _build_cell(nc, sg, iota, li, rec, w4, H, rcols,
                                       t0, Tq)
                    for j in range(t0, t0 + Tq):
                        o_tiles[j] = (o, r, t0)
            # window pairs share a PSUM bank
            ws = B["ws"]
            for i0 in range(0, len(ws), 2):
                pair = ws[i0:i0 + 2]
                npair = len(pair)
                pw = ps.tile([P, npair, rcols], f32, tag="pw")
                for k, w in enumerate(pair):
                    _win_matmuls(nc, m, b, w, o_tiles, pw[:, k, :], 0, rcols)
                hf = _alpha_elu(nc, se, pw, bt, H, C, npair, "1")
                # --- inlined phase2 per window ---
                for k, w in enumerate(pair):
                    hT = ps2.tile([P, P], bf16, tag="hT")
                    nc.tensor.matmul(out=hT[:], lhsT=hf[:, k, :],
                                     rhs=ident[:], is_transpose=True,
                                     start=True, stop=True)
                    hTs = se.tile([P, P], bf16, tag="hTs")
                    nc.scalar.activation(out=hTs[:], in_=hT[:], func=AF.Copy)
                    h2p = ps2.tile([P, C], f32, tag="h2p")
                    nc.tensor.matmul(out=h2p[:], lhsT=hTs[:], rhs=W2t[:],
                                     start=True, stop=True)
                    rec2 = se.tile([P, P], bf16, tag="rec2")
                    nc.vector.memset(rec2[:, C:], 0.0)
                    nc.vector.memset(rec2[:, C:C + 1], 1.0)
                    nc.scalar.activation(out=rec2[:, 0:C], in_=h2p[:],
                                         func=AF.Copy)
                    prod2 = se.tile([P, 2, C], bf16, tag="prod2")
                    nc.vector.tensor_tensor(
                        out=prod2[:],
                        in0=rec2[:, 0:C].unsqueeze(1).to_broadcast([P, 2, C]),
                        in1=a2t[:].rearrange("p (k c) -> p k c", k=2),
                        op=AL.mult)
                    sd2 = se.tile([P, 2], f32, tag="sd2")
                    nc.vector.tensor_reduce(
                        out=sd2[:], in_=prod2[:], axis=mybir.AxisListType.X,
                        op=AL.add)
                    nc.vector.tensor_copy(
                        out=rec2[:, C + 2:C + 4].bitcast(f32), in_=sd2[:, 0:1])
                    nc.sync.dma_start(out=cc2[w * P:(w + 1) * P, :], in_=rec2[:])
                    d2 = se.tile([P, 1], f32, tag="d2")
                    nc.vector.tensor_copy(out=d2[:], in_=sd2[:, 1:2])
                    nc.sync.dma_start(out=d2loc[w * P:(w + 1) * P, 0:1], in_=d2[:])
                    # chunked AllGather of cc2 as windows complete
                    while (ag_done + 1) * WPCH <= w + 1:
                        k2 = ag_done
                        nc.gpsimd.collective_compute(
                            kind="AllGather", op=AL.bypass, replica_groups=rg,
                            ins=[cc2[k2 * m.CR:(k2 + 1) * m.CR, :]],
                            outs=[table2[k2 * m.CR * NCORES:
                                         (k2 + 1) * m.CR * NCORES, :]])
                        ag_done += 1
        dum = sbc.tile([1, 1], f32)
        nc.vector.memset(dum[:], -300.0)
        nc.sync.dma_start(out=d2loc[m.NPC_pad:m.NPC_pad + 1, 0:1], in_=dum[:])


def _gat_layer2(nc, tc, m, table2, d2loc, b2_bc, iota_bc, rec_idx, d_idx,
                li_in, gidA_in, gidB_in, recip_in, Wlin, blin,
                po_in, po_out, out_t, rg):
    """L2 edge loop + inlined graph pooling + final linear."""
    C = m.hid
    AL = mybir.AluOpType
    AF = mybir.ActivationFunctionType
    R2 = C + 4
    rcols = C + 1                    # 33
    s_off = C + 2
    rec_col_off = np.cumsum([0] + m.rec_cols)
    d_col_off = np.cumsum([0] + m.d_cols)
    li_col_off = np.cumsum([0] + m.li_cols)

    with tc.tile_pool(name="L2", bufs=2) as sb, \
         tc.tile_pool(name="L2c", bufs=1) as sbc, \
         tc.tile_pool(name="L2g", bufs=6) as sg, \
         tc.tile_pool(name="L2e", bufs=2) as se, \
         tc.tile_pool(name="L2ps", bufs=2, space="PSUM") as ps, \
         tc.tile_pool(name="L2pool", bufs=1, space="PSUM") as pp, \
         tc.tile_pool(name="L2fin", bufs=1, space="PSUM") as pf:
        iota = sbc.tile([P, P], bf16)
        nc.sync.dma_start(out=iota[:], in_=iota_bc[:, :])
        bt = sbc.tile([P, C], f32)
        nc.sync.dma_start(out=bt[:], in_=b2_bc[:, 0:C])
        gA = sbc.tile([P, m.NT], bf16)
        nc.sync.dma_start(out=gA[:], in_=gidA_in[:, :])
        gB = sbc.tile([P, m.NT], bf16)
        nc.sync.dma_start(out=gB[:], in_=gidB_in[:, :])
        pA = pp.tile([P, C], f32, tag="pA")
        pB = pp.tile([P, C], f32, tag="pB")

        for b in range(m.NB):
            B = m.batches[b]
            Tb = B["Tb"]
            if Tb == 0:
                continue
            li, rec, dg = _load_batch(
                nc, sb, m, b, table2, d2loc, R2, 1, rec_idx, d_idx, li_in,
                rec_col_off, d_col_off, li_col_off)
            w4 = _edge_weights(nc, sb, rec, dg, s_off, 1, Tb)
            o_tiles = {}
            for w in B["ws"]:
                for q in range(4):
                    Tq = int(m.Twq[w, q])
                    if Tq == 0:
                        continue
                    t0 = B["blk"][(w, q)]
                    o, r = _build_cell(nc, sg, iota, li, rec, w4, 1, rcols,
                                       t0, Tq)
                    for j in range(t0, t0 + Tq):
                        o_tiles[j] = (o, r, t0)
            ws = B["ws"]
            for i0 in range(0, len(ws), 2):
                pair = ws[i0:i0 + 2]
                npair = len(pair)
                pw = ps.tile([P, npair, rcols], f32, tag="pw")
                for k, w in enumerate(pair):
                    _win_matmuls(nc, m, b, w, o_tiles, pw[:, k, :], 0, rcols)
                hf = _alpha_elu(nc, se, pw, bt, 1, C, npair, "2")
                # --- inlined pooling per window ---
                for k, w in enumerate(pair):
                    for g_t, acc in ((gA, pA), (gB, pB)):
                        O = se.tile([P, P], bf16, tag="Opool")
                        nc.vector.tensor_tensor(
                            out=O[:], in0=iota[:],
                            in1=g_t[:, w:w + 1].to_broadcast([P, P]),
                            op=AL.is_equal)
                        nc.tensor.matmul(out=acc[:], lhsT=O[:],
                                         rhs=hf[:, k, :],
                                         start=(w == 0), stop=(w == m.NT - 1),
                                         skip_group_check=True)
        # ---- pooled sums -> AllReduce -> mean -> linear
        sA = sb.tile([P, C], f32)
        nc.vector.tensor_copy(out=sA[:], in_=pA[:])
        sB = sb.tile([P, C], f32)
        nc.vector.tensor_copy(out=sB[:], in_=pB[:])
        nc.sync.dma_start(out=po_in[0:P, :], in_=sA[:])
        nc.sync.dma_start(out=po_in[P:256, :], in_=sB[:])
        nc.gpsimd.collective_compute(
            kind="AllReduce", op=AL.add, replica_groups=rg,
            ins=[po_in[:, :]], outs=[po_out[:, :]])
        rcp = sbc.tile([P, 2], f32)
        nc.sync.dma_start(out=rcp[:], in_=recip_in[:, :])
        identf = sbc.tile([P, P], f32)
        make_identity(nc, identf[:])
        WT = sbc.tile([C, 10], f32)
        nc.sync.dma_start(out=WT[:], in_=Wlin[:, :])
        bl = sbc.tile([10, 1], f32)
        nc.sync.dma_start(out=bl[:], in_=blin[:, :])
        poT = sb.tile([C, 256], f32)
        for half in range(2):
            pm = sb.tile([P, C], f32, tag="pm")
            nc.sync.dma_start(out=pm[:], in_=po_out[half * P:(half + 1) * P, :])
            nc.vector.tensor_scalar(
                out=pm[:], in0=pm[:], scalar1=rcp[:, half:half + 1],
                scalar2=None, op0=AL.mult)
            tp = pf.tile([C, P], f32, tag="tp")
            nc.tensor.transpose(out=tp[:], in_=pm[:], identity=identf[:])
            nc.vector.tensor_copy(out=poT[:, half * P:(half + 1) * P], in_=tp[:])
        om = pf.tile([10, 256], f32, tag="om")
        nc.tensor.matmul(out=om[:], lhsT=WT[:], rhs=poT[:], start=True, stop=True)
        ob = sb.tile([10, 256], f32)
        nc.scalar.activation(out=ob[:], in_=om[:], func=AF.Identity, bias=bl[:, 0:1])
        for half in range(2):
            tp2 = pf.tile([P, 10], f32, tag="tp2")
            nc.tensor.transpose(out=tp2[:], in_=ob[:, half * P:(half + 1) * P],
                                identity=identf[0:10, 0:10])
            oo = sb.tile([P, 10], f32, tag="oo")
            nc.vector.tensor_copy(out=oo[:], in_=tp2[:])
            nc.sync.dma_start(out=out_t[half * P:(half + 1) * P, :], in_=oo[:])


# ---------------------------------------------------------------- entry point

def kernel(x, edge_index, batch, W1, a_src1, a_dst1, b1, W2, a_src2, a_dst2,
           b2, W_lin, b_lin):
    global _last_exec_ns
    x = np.asarray(x)
    N, IN_C = x.shape
    heads, hid = np.asarray(a_src1).shape
    m = _host_prep(x, np.asarray(edge_index), np.asarray(batch), heads, hid)

    nc = _build(m)

    bfl = ml_dtypes.bfloat16
    HC = heads * hid
    in_maps = []
    a1 = np.concatenate([np.asarray(a_src1).reshape(-1),
                         np.asarray(a_dst1).reshape(-1)]).astype(bfl)
    a2 = np.concatenate([np.asarray(a_src2).reshape(-1),
                         np.asarray(a_dst2).reshape(-1)]).astype(bfl)
    iota = np.tile(np.arange(P, dtype=np.float64), (P, 1)).astype(bfl)
    recip2 = np.stack([m.recip[0:P], m.recip[P:256]], 1).astype(np.float32)
    for c in range(NCORES):
        pc = m.per_core[c]
        xs = np.zeros((m.NPC_pad, HC), np.float32)
        xs[0:m.NPC] = x[c * m.NPC:(c + 1) * m.NPC]
        xT = np.ascontiguousarray(xs.T).astype(bfl)
        in_maps.append({
            "xT_sl": xT,
            "W1b": np.asarray(W1).astype(bfl),
            "a1_bc": np.tile(a1, (P, 1)),
            "b1_bc": np.tile(np.asarray(b1).reshape(1, -1), (P, 1)).astype(np.float32),
            "W2b": np.asarray(W2).astype(bfl),
            "a2_bc": np.tile(a2, (P, 1)),
            "b2_bc": np.tile(np.asarray(b2).reshape(1, -1), (P, 1)).astype(np.float32),
            "Wlin": np.asarray(W_lin).astype(np.float32),
            "blin": np.asarray(b_lin).reshape(10, 1).astype(np.float32),
            "recip_in": recip2,
            "iota_bc": iota,
            "rec_idx": pc["rec_idx"],
            "d_idx": pc["d_idx"],
            "li_in": pc["li"],
            "gidA": pc["gidA"],
            "gidB": pc["gidB"],
        })

    import os
    if os.environ.get("GAT_SIM"):
        from concourse.bass_interp import MultiCoreSim
        mcs = MultiCoreSim(nc, NCORES, require_finite=False, require_nnan=False)
        for c in range(NCORES):
            core = mcs.cores[c]
            for k, v in in_maps[c].items():
                core.tensor(k)[:] = v
        mcs.simulate()
        return np.ascontiguousarray(np.asarray(mcs.cores[0].mem_tensor("out")))

    want_trace = bool(os.environ.get("GAT_TRACE"))
    if want_trace:
        _install_ntff_hook()
    try:
        res = run_bass_kernel_spmd(nc, in_maps, core_ids=list(range(NCORES)),
                                   trace=want_trace)
    except ModuleNotFoundError:
        res = run_bass_kernel_spmd(nc, in_maps, core_ids=list(range(NCORES)),
                                   trace=False)
    _last_exec_ns = res.exec_time_ns
    return np.ascontiguousarray(res.results[0]["out"])


def run(x, edge_index, batch, W1, a_src1, a_dst1, b1, W2, a_src2, a_dst2,
        b2, W_lin, b_lin):
    return kernel(x, edge_index, batch, W1, a_src1, a_dst1, b1, W2, a_src2,
                  a_dst2, b2, W_lin, b_lin)
